# revision 34
# baseline (speedup 1.0000x reference)
"""Distributed Trainium2 (Bass/Tile) kernel for the AdaMEOW GNN loss.

Sharding: target-node dim N row-sharded across 8 cores (128 rows each);
neighbor dim M sharded (512 each) for the neighbor-feature MLPs, combined
with one ReduceScatter.  GCN activations ([N,64] per layer) exchanged with
two AllGathers; a third AllGather shares projected z_coarse + attention
partials.  The [N,N,E] InfoNCE pair tensor is never materialized: the
pair-MLP is fused as w[i,j] = sigmoid(sum_h tanh(A[i,h]+B[j,h])*m2[h]+b2).

Matmul operands are bf16 (PE fp32 runs at 1/4 rate; bf16 also halves the
input DMA bytes); all accumulation is f32 in PSUM, and the softmax / l2norm
/ exp / log chains stay f32.
"""

import os

import ml_dtypes
import numpy as np

import concourse.bass as bass
import concourse.mybir as mybir
import concourse.tile as tile
from concourse import bacc
from concourse.bass_utils import run_bass_kernel_spmd

FP = mybir.dt.float32
BF = mybir.dt.bfloat16
NPBF = ml_dtypes.bfloat16
AF = mybir.ActivationFunctionType
ALU = mybir.AluOpType

N, M, D0, D1, H, E = 1024, 4096, 1024, 512, 512, 64
C = 8            # cores
NL = N // C      # 128 local target nodes
ML = M // C      # 512 local neighbor nodes
P = 128
HK = H // P      # 4
D0K = D0 // P    # 8
MLK = ML // P    # 4
NB = N // P      # 8 node blocks
TAU = 0.5
RG = [list(range(C))]

RS_ROWS = 1032   # 2x512 aggT rows + 2 cnt rows + 6 pad


def _build():
    nc = bacc.Bacc("TRN2", num_devices=C)

    def din(name, shape, dt=BF):
        return nc.declare_dram_parameter(name, list(shape), dt, isOutput=False)

    # per-core sharded inputs (host pre-transposed, bf16)
    feat0T = din("feat0T", (D0, NL))
    maskT = din("maskT", (D0, NL))
    feat1T = din("feat1T", (D1, ML))
    feat2T = din("feat2T", (D1, ML))
    nei0T = din("nei0T", (ML, N))
    nei1T = din("nei1T", (ML, N))
    adj0T = din("adj0T", (N, NL))
    adj1T = din("adj1T", (N, NL))
    madj0T = din("madj0T", (N, NL))
    madj1T = din("madj1T", (N, NL))
    # replicated weights (bf16 for matmul operands)
    fc0_w = din("fc0_w", (D0, H))
    fc1_w = din("fc1_w", (D1, H))
    fc2_w = din("fc2_w", (D1, H))
    agg0_w = din("agg0_w", (H, H))
    agg1_w = din("agg1_w", (H, H))
    gcn_w1 = din("gcn_w1", (H, E))
    gcn_w2 = din("gcn_w2", (E, E))
    att_w = din("att_w", (E, E))
    proj_w = din("proj_w", (E, E))
    mlp1_w = din("mlp1_w", (E, 16))
    sel16 = din("sel16", (16, 16 * P))   # host constant: eye16 (x) ones(1,P)
    # small aux tensors stay f32
    mlp2_w = din("mlp2_w", (1, 16), FP)  # host passes mlp2_w.T
    fc0_b = din("fc0_b", (P, HK), FP)    # feature-partition layout
    fc1_b = din("fc1_b", (1, H), FP)     # row layout (broadcast over parts)
    fc2_b = din("fc2_b", (1, H), FP)
    gcn_b1 = din("gcn_b1", (E, 1), FP)
    gcn_b2 = din("gcn_b2", (E, 1), FP)
    att_b = din("att_b", (E, 1), FP)
    att_vec = din("att_vec", (E, 1), FP)
    proj_b = din("proj_b", (E, 1), FP)
    mlp1_b = din("mlp1_b", (1, 16), FP)
    mlp2_b = din("mlp2_b", (1, 1), FP)

    out_ext = nc.declare_dram_parameter("out", [1, 1], FP, isOutput=True)

    # collective bounce buffers (bf16 payloads)
    rs_in = nc.dram_tensor("rs_in", [NB, RS_ROWS, P], BF)
    rs_out = nc.dram_tensor("rs_out", [RS_ROWS, P], BF)
    ag1_in = nc.dram_tensor("ag1_in", [5 * P, E], BF)
    ag1_out = nc.dram_tensor("ag1_out", [C * 5 * P, E], BF, addr_space="Shared")
    ag2_in = nc.dram_tensor("ag2_in", [5 * P, E], BF)
    ag2_out = nc.dram_tensor("ag2_out", [C * 5 * P, E], BF, addr_space="Shared")
    ag3_in = nc.dram_tensor("ag3_in", [E + 1, P], BF)
    ag3_out = nc.dram_tensor("ag3_out", [C * (E + 1), P], BF,
                             addr_space="Shared")

    with tile.TileContext(nc) as tc:
        with (
            tc.tile_pool(name="pers", bufs=1) as pers,
            tc.tile_pool(name="wk512", bufs=4) as wk512,
            tc.tile_pool(name="wk128", bufs=6) as wk128,
            tc.tile_pool(name="wksm", bufs=3) as wksm,
            tc.tile_pool(name="psA", bufs=3, space="PSUM") as psA,
            tc.tile_pool(name="psB", bufs=5, space="PSUM") as psB,
        ):
            def mk(pool, shape, name, dt=FP):
                return pool.tile(list(shape), dt, tag=name, name=name)

            def w128(name="t128"):
                return wk128.tile([P, P], FP, tag=name, name=name)

            def wsm(shape, name="tsm"):
                return wksm.tile(list(shape), FP, tag=name, name=name)

            def ld(pool, dram, shape, name, pat=None, eng=None, **kw):
                t = mk(pool, shape, name, dt=dram.dtype)
                src = dram[:] if pat is None else dram[:].rearrange(pat, **kw)
                (eng or nc.sync).dma_start(t[:], src)
                return t

            # ---------------- persistent constants / small weights ----
            ones_col = mk(pers, (P, 1), "ones_col", BF)
            nc.vector.memset(ones_col[:], 1.0)
            ones_row = mk(pers, (1, P), "ones_row", BF)
            nc.vector.memset(ones_row[:], 1.0)
            onesf_col = mk(pers, (P, 1), "onesf_col", FP)
            nc.vector.memset(onesf_col[:], 1.0)

            gcnw1_sb = ld(pers, gcn_w1, (P, HK, E), "gcnw1",
                          "(o p) f -> p o f", p=P)
            gcnw2_sb = ld(pers, gcn_w2, (E, E), "gcnw2")
            attw_sb = ld(pers, att_w, (E, E), "attw")
            projw_sb = ld(pers, proj_w, (E, E), "projw")
            mlp1w_sb = ld(pers, mlp1_w, (E, 16), "mlp1w")
            gcnb1_sb = ld(pers, gcn_b1, (E, 1), "gcnb1")
            gcnb2_sb = ld(pers, gcn_b2, (E, 1), "gcnb2")
            attb_sb = ld(pers, att_b, (E, 1), "attb")
            attv_sb = ld(pers, att_vec, (E, 1), "attv")
            projb_sb = ld(pers, proj_b, (E, 1), "projb")
            mlp1b_row = ld(pers, mlp1_b, (1, 16), "mlp1b")
            b2bc_sb = mk(pers, (P, 1), "b2bc")
            nc.sync.dma_start(b2bc_sb[:], mlp2_b[:].to_broadcast((P, 1)))

            htarT_sb = mk(pers, (P, HK, NL), "htarT", BF)
            hmaskT_sb = mk(pers, (P, HK, NL), "hmaskT", BF)
            zcT_sb = mk(pers, (E, NL), "zcT", BF)
            zfT_sb = mk(pers, (E, NL), "zfT", BF)
            zfineT = mk(pers, (E, NL), "zfineT", BF)
            cnt_sb = mk(pers, (1, 2 * P), "cnt", BF)  # [cnt0|cnt1] on part 0

            # ---------------- helpers ---------------------------------
            def elu_from(x_ap, bias_ap, out_ap, fdim):
                """out = elu(x + bias); x may be PSUM; out may be bf16."""
                r = wk512.tile([x_ap.shape[0], fdim], FP, tag="elu_r",
                               name="elu_r")
                m = wk512.tile([x_ap.shape[0], fdim], FP, tag="elu_m",
                               name="elu_m")
                if bias_ap is None:
                    nc.vector.tensor_scalar_max(r[:], x_ap, 0.0)
                    nc.vector.tensor_scalar_min(m[:], x_ap, 0.0)
                else:
                    nc.vector.tensor_scalar(
                        out=r[:], in0=x_ap, scalar1=bias_ap, scalar2=0.0,
                        op0=ALU.add, op1=ALU.max)
                    nc.vector.tensor_scalar(
                        out=m[:], in0=x_ap, scalar1=bias_ap, scalar2=0.0,
                        op0=ALU.add, op1=ALU.min)
                nc.scalar.activation(m[:], m[:], AF.Exp)
                # out = (exp(min(x,0)) - 1) + max(x,0)
                nc.vector.scalar_tensor_tensor(
                    out=out_ap, in0=m[:], scalar=-1.0, in1=r[:],
                    op0=ALU.add, op1=ALU.add)

            # ================= stage 1 (scoped pool) ==================
            with tc.tile_pool(name="s1", bufs=1) as s1:
                # spread the big input DMAs over both DGE paths
                feat1T_sb = ld(s1, feat1T, (P, MLK, ML), "feat1T",
                               "(o p) f -> p o f", p=P, eng=nc.gpsimd)
                feat2T_sb = ld(s1, feat2T, (P, MLK, ML), "feat2T",
                               "(o p) f -> p o f", p=P, eng=nc.sync)
                fc1w_sb = ld(s1, fc1_w, (P, HK, H), "fc1w",
                             "(o p) f -> p o f", p=P, eng=nc.gpsimd)
                fc2w_sb = ld(s1, fc2_w, (P, HK, H), "fc2w",
                             "(o p) f -> p o f", p=P, eng=nc.sync)
                nei0T_sb = ld(s1, nei0T, (P, MLK, N), "nei0T",
                              "(o p) f -> p o f", p=P, eng=nc.gpsimd)
                nei1T_sb = ld(s1, nei1T, (P, MLK, N), "nei1T",
                              "(o p) f -> p o f", p=P, eng=nc.sync)
                fc1b_bc = mk(s1, (P, H), "fc1b_bc")
                nc.gpsimd.dma_start(fc1b_bc[:], fc1_b[:].to_broadcast((P, H)))
                fc2b_bc = mk(s1, (P, H), "fc2b_bc")
                nc.gpsimd.dma_start(fc2b_bc[:], fc2_b[:].to_broadcast((P, H)))
                fc0w_sb = ld(s1, fc0_w, (P, D0K, H), "fc0w",
                             "(o p) f -> p o f", p=P, eng=nc.gpsimd)
                feat0T_sb = ld(s1, feat0T, (P, D0K, NL), "feat0T",
                               "(o p) f -> p o f", p=P, eng=nc.sync)
                maskT_sb = ld(s1, maskT, (P, D0K, NL), "maskT",
                              "(o p) f -> p o f", p=P, eng=nc.sync)
                fc0b_sb = ld(s1, fc0_b, (P, HK), "fc0b")

                # ---- h_nei shards: elu(featX @ fcX_w + fcX_b) --------
                hnei_sb = [mk(s1, (P, MLK, H), "hnei0", BF),
                           mk(s1, (P, MLK, H), "hnei1", BF)]
                for v, (fT, fw, fbc) in enumerate(
                    [(feat1T_sb, fc1w_sb, fc1b_bc),
                     (feat2T_sb, fc2w_sb, fc2b_bc)]
                ):
                    for mc in range(MLK):
                        ps = psA.tile([P, H], FP, tag="psA", name="ps_hnei")
                        for k in range(MLK):
                            nc.tensor.matmul(
                                ps[:], fT[:, k, mc * P:(mc + 1) * P],
                                fw[:, k, :],
                                start=(k == 0), stop=(k == MLK - 1))
                        xb = wk512.tile([P, H], FP, tag="xb", name="xb")
                        nc.vector.tensor_add(xb[:], ps[:], fbc[:])
                        elu_from(xb[:], None, hnei_sb[v][:, mc, :], H)

                # ---- partial aggregation (feature-major) + counts ----
                # coalesced rs_in writes: one strided DMA per [128,512]
                # PSUM tile (covers 4 node blocks), alternating DGE rings.
                wq = [nc.sync, nc.gpsimd]
                for v, neiT in enumerate([nei0T_sb, nei1T_sb]):
                    for hc in range(HK):
                        for jh in range(2):
                            ps = psA.tile([P, 512], FP, tag="psA",
                                          name="ps_pr")
                            for k in range(MLK):
                                nc.tensor.matmul(
                                    ps[:],
                                    hnei_sb[v][:, k, hc * P:(hc + 1) * P],
                                    neiT[:, k, jh * 512:(jh + 1) * 512],
                                    start=(k == 0), stop=(k == MLK - 1))
                            prs = wk512.tile([P, 512], BF, tag="prs",
                                             name="prs")
                            nc.vector.tensor_copy(prs[:], ps[:])
                            wq[(hc + jh) % 2].dma_start(
                                rs_in[jh * 4:(jh + 1) * 4,
                                      v * H + hc * P:v * H + (hc + 1) * P,
                                      :].rearrange("b p n -> p b n"),
                                prs[:].rearrange("p (b n) -> p b n", b=4))
                    for jh in range(2):
                        psc = psB.tile([1, 512], FP, tag="psB", name="ps_cnt")
                        for k in range(MLK):
                            nc.tensor.matmul(
                                psc[:], ones_col[:],
                                neiT[:, k, jh * 512:(jh + 1) * 512],
                                start=(k == 0), stop=(k == MLK - 1))
                        cst = wksm.tile([1, 512], BF, tag="cst", name="cst")
                        nc.vector.tensor_copy(cst[:], psc[:])
                        nc.sync.dma_start(
                            rs_in[jh * 4:(jh + 1) * 4,
                                  2 * H + v:2 * H + v + 1,
                                  :].rearrange("b one n -> one b n"),
                            cst[:].rearrange("one (b n) -> one b n", b=4))
                nc.gpsimd.collective_compute(
                    "ReduceScatter", ALU.add, replica_groups=RG,
                    ins=[rs_in[:].opt()], outs=[rs_out[:].opt()])

                # ---- h_tarT / h_maskT (overlaps the ReduceScatter) ---
                for src, dst in [(feat0T_sb, htarT_sb),
                                 (maskT_sb, hmaskT_sb)]:
                    for hc in range(HK):
                        ps = psB.tile([P, NL], FP, tag="psB", name="ps_htar")
                        for k in range(D0K):
                            nc.tensor.matmul(
                                ps[:], fc0w_sb[:, k, hc * P:(hc + 1) * P],
                                src[:, k, :],
                                start=(k == 0), stop=(k == D0K - 1))
                        elu_from(ps[:], fc0b_sb[:, hc:hc + 1],
                                 dst[:, hc, :], NL)

            # ================= late pool (stage 2+) ===================
            with tc.tile_pool(name="late", bufs=1) as late:
                adj0T_sb = ld(late, adj0T, (P, NB, NL), "adj0T",
                              "(o p) f -> p o f", p=P, eng=nc.gpsimd)
                adj1T_sb = ld(late, adj1T, (P, NB, NL), "adj1T",
                              "(o p) f -> p o f", p=P, eng=nc.sync)
                madj0T_sb = ld(late, madj0T, (P, NB, NL), "madj0T",
                               "(o p) f -> p o f", p=P, eng=nc.gpsimd)
                madj1T_sb = ld(late, madj1T, (P, NB, NL), "madj1T",
                               "(o p) f -> p o f", p=P, eng=nc.sync)
                agg0w_sb = ld(late, agg0_w, (P, HK, H), "agg0w",
                              "(o p) f -> p o f", p=P, eng=nc.gpsimd)
                agg1w_sb = ld(late, agg1_w, (P, HK, H), "agg1w",
                              "(o p) f -> p o f", p=P, eng=nc.sync)

                mean_sb = mk(late, (P, NB, NL), "meanadjT", BF)
                for s in range(NB):
                    nc.vector.tensor_add(
                        mean_sb[:, s, :], adj0T_sb[:, s, :], adj1T_sb[:, s, :])

                # ---- views from RS result ----------------------------
                aggT_sb = mk(late, (P, 2 * HK, NL), "aggT", BF)
                nc.sync.dma_start(
                    aggT_sb[:],
                    rs_out[0:2 * H, :].rearrange("(o p) n -> p o n", p=P))
                nc.sync.dma_start(
                    cnt_sb[:],
                    rs_out[2 * H:2 * H + 2, :].rearrange(
                        "r n -> (r n)").rearrange("(one f) -> one f", one=1))

                xs_sb = [htarT_sb,
                         mk(late, (P, HK, NL), "x_v0", BF),
                         mk(late, (P, HK, NL), "x_m0", BF),
                         mk(late, (P, HK, NL), "x_v1", BF),
                         mk(late, (P, HK, NL), "x_m1", BF)]
                for v, aggw in enumerate([agg0w_sb, agg1w_sb]):
                    cm = wsm((1, P), "cm")
                    nc.vector.tensor_scalar_max(
                        cm[:], cnt_sb[0:1, v * P:(v + 1) * P], 1.0)
                    rec = wsm((1, P), "rec")
                    nc.vector.reciprocal(rec[:], cm[:])
                    recb = wksm.tile([1, P], BF, tag="recb", name="recb")
                    nc.vector.tensor_copy(recb[:], rec[:])
                    psb = psB.tile([P, P], FP, tag="psB", name="ps_recbc")
                    nc.tensor.matmul(psb[:], ones_row[:], recb[:])
                    rec_bc = w128("rec_bc")
                    nc.vector.tensor_copy(rec_bc[:], psb[:])
                    for hc in range(HK):
                        pst = psB.tile([P, P], FP, tag="psB", name="ps_t")
                        for k in range(HK):
                            nc.tensor.matmul(
                                pst[:], aggw[:, k, hc * P:(hc + 1) * P],
                                aggT_sb[:, v * HK + k, :],
                                start=(k == 0), stop=(k == HK - 1))
                        tsc = w128("tsc")
                        nc.vector.tensor_mul(tsc[:], pst[:], rec_bc[:])
                        for src, dst in [(htarT_sb, xs_sb[1 + 2 * v]),
                                         (hmaskT_sb, xs_sb[2 + 2 * v])]:
                            x = w128("xsum")
                            nc.vector.tensor_add(x[:], tsc[:], src[:, hc, :])
                            elu_from(x[:], None, dst[:, hc, :], NL)

                # ---- GCN ---------------------------------------------
                adjs = [mean_sb, adj0T_sb, madj0T_sb, adj1T_sb, madj1T_sb]

                st5a = mk(late, (P, 5, E), "st5a", BF)
                for gi in range(5):
                    ps = psB.tile([P, E], FP, tag="psB", name="ps_y1")
                    for k in range(HK):
                        nc.tensor.matmul(
                            ps[:], xs_sb[gi][:, k, :], gcnw1_sb[:, k, :],
                            start=(k == 0), stop=(k == HK - 1))
                    if gi == 0:
                        nc.vector.tensor_scalar_mul(
                            st5a[:, gi, :], ps[:], 0.5)
                    else:
                        nc.vector.tensor_copy(st5a[:, gi, :], ps[:])
                nc.sync.dma_start(
                    ag1_in[:].rearrange("(g p) e -> p g e", p=P), st5a[:])

                nc.gpsimd.collective_compute(
                    "AllGather", ALU.bypass, replica_groups=RG,
                    ins=[ag1_in[:].opt()], outs=[ag1_out[:].opt()])

                y1_sb = mk(late, (P, 5 * C, E), "y1", BF)
                y1src = ag1_out[:].rearrange("(o p) e -> p o e", p=P)
                nc.sync.dma_start(y1_sb[:, 0:20, :], y1src[:, 0:20, :])
                nc.gpsimd.dma_start(y1_sb[:, 20:40, :], y1src[:, 20:40, :])

                h_sb = mk(late, (E, 5, NL), "hT", BF)
                for gi in range(5):
                    ps = psB.tile([E, NL], FP, tag="psB", name="ps_h")
                    for s in range(NB):
                        nc.tensor.matmul(
                            ps[:], y1_sb[:, s * 5 + gi, :], adjs[gi][:, s, :],
                            start=(s == 0), stop=(s == NB - 1))
                    nc.vector.tensor_scalar(
                        out=h_sb[:, gi, :], in0=ps[:], scalar1=gcnb1_sb[:],
                        scalar2=0.0, op0=ALU.add, op1=ALU.max)

                st5b = mk(late, (P, 5, E), "st5b", BF)
                for gi in range(5):
                    ps = psB.tile([P, E], FP, tag="psB", name="ps_y2")
                    nc.tensor.matmul(ps[:], h_sb[:, gi, :], gcnw2_sb[:])
                    if gi == 0:
                        nc.vector.tensor_scalar_mul(
                            st5b[:, gi, :], ps[:], 0.5)
                    else:
                        nc.vector.tensor_copy(st5b[:, gi, :], ps[:])
                nc.sync.dma_start(
                    ag2_in[:].rearrange("(g p) e -> p g e", p=P), st5b[:])

                nc.gpsimd.collective_compute(
                    "AllGather", ALU.bypass, replica_groups=RG,
                    ins=[ag2_in[:].opt()], outs=[ag2_out[:].opt()])

                y2_sb = mk(late, (P, 5 * C, E), "y2", BF)
                y2src = ag2_out[:].rearrange("(o p) e -> p o e", p=P)
                nc.sync.dma_start(y2_sb[:, 0:20, :], y2src[:, 0:20, :])
                nc.gpsimd.dma_start(y2_sb[:, 20:40, :], y2src[:, 20:40, :])

                zT_sb = mk(late, (E, 5, NL), "zT", BF)
                for gi in range(5):
                    ps = psB.tile([E, NL], FP, tag="psB", name="ps_z")
                    for s in range(NB):
                        nc.tensor.matmul(
                            ps[:], y2_sb[:, s * 5 + gi, :], adjs[gi][:, s, :],
                            start=(s == 0), stop=(s == NB - 1))
                    nc.vector.tensor_scalar_add(
                        zT_sb[:, gi, :], ps[:], gcnb2_sb[:])

                def colnorm_scale(src_ap, out_ap):
                    """out = src / max(||src||_col, 1e-12) column-wise."""
                    sqb = wksm.tile([E, NL], BF, tag="sqb", name="sqb")
                    nc.vector.tensor_mul(sqb[:], src_ap, src_ap)
                    psn = psB.tile([1, NL], FP, tag="psB", name="ps_norm")
                    nc.tensor.matmul(psn[:], ones_col[0:E, :], sqb[:])
                    nr = wsm((1, NL), "nr")
                    nc.scalar.activation(nr[:], psn[:], AF.Sqrt)
                    nc.vector.tensor_scalar_max(nr[:], nr[:], 1e-12)
                    rc = wsm((1, NL), "rc")
                    nc.vector.reciprocal(rc[:], nr[:])
                    rcb = wksm.tile([1, NL], BF, tag="rcb", name="rcb")
                    nc.vector.tensor_copy(rcb[:], rc[:])
                    psb2 = psB.tile([P, NL], FP, tag="psB", name="ps_nbc")
                    nc.tensor.matmul(psb2[:], ones_row[:], rcb[:])
                    nc.vector.tensor_mul(out_ap, src_ap, psb2[0:E, :])

                # batched l2norm of the 4 fine views: [E, 4*NL] at once
                hsT_sb = mk(late, (E, 4, NL), "hsT", BF)
                z4 = zT_sb[:, 1:5, :]
                sq4 = wksm.tile([E, 4 * NL], BF, tag="sq4", name="sq4")
                nc.vector.tensor_mul(sq4[:], z4, z4)
                psn4 = psA.tile([1, 4 * NL], FP, tag="psA", name="ps_n4")
                nc.tensor.matmul(psn4[:], ones_col[0:E, :], sq4[:])
                nr4 = wksm.tile([1, 4 * NL], FP, tag="nr4", name="nr4")
                nc.scalar.activation(nr4[:], psn4[:], AF.Sqrt)
                nc.vector.tensor_scalar_max(nr4[:], nr4[:], 1e-12)
                rc4f = wksm.tile([1, 4 * NL], FP, tag="rc4f", name="rc4f")
                nc.vector.reciprocal(rc4f[:], nr4[:])
                rc4 = wksm.tile([1, 4 * NL], BF, tag="rc4", name="rc4")
                nc.vector.tensor_copy(rc4[:], rc4f[:])
                psb4 = psA.tile([P, 4 * NL], FP, tag="psA", name="ps_nb4")
                nc.tensor.matmul(psb4[:], ones_row[:], rc4[:])
                nc.vector.tensor_mul(hsT_sb[:], z4, psb4[0:E, :])

                # attention partials, batched: attw.T @ [hs0|hs1|hs2|hs3]
                psa4 = psA.tile([E, 4 * NL], FP, tag="psA", name="ps_att4")
                nc.tensor.matmul(psa4[:], attw_sb[:], hsT_sb[:])
                ta4 = wsm((E, 4 * NL), "ta4")
                nc.scalar.activation(ta4[:], psa4[:], AF.Tanh,
                                     bias=attb_sb[:])
                mm4 = wsm((E, 4 * NL), "mm4")
                nc.vector.tensor_scalar(
                    out=mm4[:], in0=ta4[:], scalar1=attv_sb[:], scalar2=0.0,
                    op0=ALU.mult, op1=ALU.add)
                reds = wsm((E, 4), "reds")
                nc.vector.reduce_sum(
                    reds[:], mm4[:].rearrange("e (v n) -> e v n", v=4),
                    axis=mybir.AxisListType.X)
                redsb = wksm.tile([E, 4], BF, tag="redsb", name="redsb")
                nc.vector.tensor_copy(redsb[:], reds[:])
                pse = psB.tile([1, 4], FP, tag="psB", name="ps_e")
                nc.tensor.matmul(pse[:], ones_col[0:E, :], redsb[:])
                e_row = wksm.tile([1, P], BF, tag="e_row", name="e_row")
                nc.vector.memset(e_row[:], 0.0)
                nc.vector.tensor_scalar_mul(e_row[:, 0:4], pse[:], 1.0 / N)

                def proj_norm(src_ap, dst):
                    ps = psB.tile([E, NL], FP, tag="psB", name="ps_proj")
                    nc.tensor.matmul(ps[:], projw_sb[:], src_ap)
                    tf = wsm((E, NL), "projt")
                    nc.scalar.activation(tf[:], ps[:], AF.Tanh,
                                         bias=projb_sb[:])
                    colnorm_scale(tf[:], dst[:])

                proj_norm(zT_sb[:, 0, :], zcT_sb)
                nc.sync.dma_start(ag3_in[0:E, :], zcT_sb[:])
                nc.sync.dma_start(ag3_in[E:E + 1, :], e_row[:])

                nc.gpsimd.collective_compute(
                    "AllGather", ALU.bypass, replica_groups=RG,
                    ins=[ag3_in[:].opt()], outs=[ag3_out[:].opt()])

                zcall_sb = mk(late, (E, C, P), "zcall", BF)
                zsrc = ag3_out[:].rearrange("(s p) n -> p s n", p=E + 1)[0:E]
                nc.sync.dma_start(zcall_sb[:, 0:4, :], zsrc[:, 0:4, :])
                nc.gpsimd.dma_start(zcall_sb[:, 4:8, :], zsrc[:, 4:8, :])
                e8_sb = wksm.tile([C, 4], BF, tag="e8", name="e8")
                nc.sync.dma_start(
                    e8_sb[:],
                    ag3_out[:].rearrange("(s p) n -> s p n", p=E + 1)[:, E, 0:4])

                # softmax over the 4 views
                pse2 = psB.tile([1, 4], FP, tag="psB", name="ps_e2")
                nc.tensor.matmul(pse2[:], ones_col[0:C, :], e8_sb[:])
                ee = wsm((1, 4), "ee")
                nc.scalar.activation(ee[:], pse2[:], AF.Exp)
                se = wsm((1, 1), "se")
                nc.vector.reduce_sum(se[:], ee[:], axis=mybir.AxisListType.X)
                nc.vector.reciprocal(se[:], se[:])
                beta_row = wksm.tile([1, 4], BF, tag="beta", name="beta")
                nc.vector.tensor_scalar_mul(beta_row[:], ee[:], se[:])
                psbb = psB.tile([P, 4], FP, tag="psB", name="ps_beta")
                nc.tensor.matmul(psbb[:], ones_row[:], beta_row[:])
                beta_bc = wsm((P, 4), "beta_bc")
                nc.vector.tensor_copy(beta_bc[:], psbb[:])

                nc.vector.tensor_scalar_mul(
                    zfineT[:], hsT_sb[:, 0, :], beta_bc[0:E, 0:1])
                for v in range(1, 4):
                    nc.vector.scalar_tensor_tensor(
                        out=zfineT[:], in0=hsT_sb[:, v, :],
                        scalar=beta_bc[0:E, v:v + 1], in1=zfineT[:],
                        op0=ALU.mult, op1=ALU.add)

                proj_norm(zfineT[:], zfT_sb)

                # ---- InfoNCE -----------------------------------------
                prod = wsm((E, NL), "prod")
                nc.vector.tensor_mul(prod[:], zfT_sb[:], zcT_sb[:])
                prodb = wksm.tile([E, NL], BF, tag="prodb", name="prodb")
                nc.vector.tensor_copy(prodb[:], prod[:])
                psd = psB.tile([NL, 1], FP, tag="psB", name="ps_diag")
                nc.tensor.matmul(psd[:], prodb[:], ones_col[0:E, :])
                diag_sb = wsm((NL, 1), "diag")
                nc.vector.tensor_scalar_mul(diag_sb[:], psd[:], 1.0 / TAU)

                dots_sb = mk(late, (P, N), "dots")
                for jh in range(2):
                    psl = psA.tile([P, 512], FP, tag="psA", name="ps_log")
                    nc.tensor.matmul(
                        psl[:], zfT_sb[:], zcall_sb[:, jh * 4:(jh + 1) * 4, :])
                    nc.scalar.activation(
                        dots_sb[:, jh * 512:(jh + 1) * 512], psl[:], AF.Exp,
                        scale=1.0 / TAU)

                # A = zf @ mlp1_w + mlp1_b   [NL, 16]
                psa2 = psB.tile([NL, 16], FP, tag="psB", name="ps_A")
                nc.tensor.matmul(psa2[:], zfT_sb[:], mlp1w_sb[:])
                b1bc = wsm((P, 16), "b1bc")
                nc.sync.dma_start(b1bc[:], mlp1_b[:].to_broadcast((P, 16)))
                A_sb = mk(late, (NL, 16), "A")
                nc.vector.tensor_add(A_sb[:], psa2[:], b1bc[0:NL, :])

                # B^T = mlp1_w^T @ zc_all   [16, N]
                BT_sb = mk(late, (16, N), "BT", BF)
                for s in range(C):
                    pst = psB.tile([16, P], FP, tag="psB", name="ps_BT")
                    nc.tensor.matmul(pst[:], mlp1w_sb[:], zcall_sb[:, s, :])
                    nc.vector.tensor_copy(BT_sb[:, s * P:(s + 1) * P], pst[:])

                # m2 broadcast [P, 16] (via DMA, f32)
                m2bc = wsm((P, 16), "m2bc")
                nc.sync.dma_start(m2bc[:], mlp2_w[:].to_broadcast((P, 16)))

                sel_sb = ld(late, sel16, (16, 16 * P), "sel")

                # acc = sum_h tanh(A[:,h] + B[j,h]) * m2[h]
                # two independent accumulators per half so the DVE
                # multiply-add chain is 8 deep instead of 16.
                acc_sb = mk(late, (P, N), "acc")
                acc2_sb = mk(late, (P, N), "acc2")
                accs = [acc_sb, acc2_sb]
                for h in range(16):
                    for jh in range(2):
                        psbt = psA.tile([P, 512], FP, tag="psA", name="ps_bbc")
                        nc.tensor.matmul(
                            psbt[:], sel_sb[:, h * P:(h + 1) * P],
                            BT_sb[:, jh * 512:(jh + 1) * 512])
                        th = wk512.tile([P, 512], FP, tag="th", name="th")
                        nc.scalar.activation(
                            th[:], psbt[:], AF.Tanh, bias=A_sb[:, h:h + 1])
                        dst = accs[h % 2][:, jh * 512:(jh + 1) * 512]
                        if h < 2:
                            nc.vector.tensor_scalar_mul(
                                dst, th[:], m2bc[:, h:h + 1])
                        else:
                            nc.vector.scalar_tensor_tensor(
                                out=dst, in0=th[:], scalar=m2bc[:, h:h + 1],
                                in1=dst, op0=ALU.mult, op1=ALU.add)
                nc.vector.tensor_add(acc_sb[:], acc_sb[:], acc2_sb[:])

                w_sb = mk(late, (P, N), "w")
                nc.scalar.activation(w_sb[:], acc_sb[:], AF.Sigmoid,
                                     bias=b2bc_sb[:])

                # denom = sum_j dots * w
                denom_sb = wsm((P, 1), "denom")
                nc.vector.scalar_tensor_tensor(
                    out=acc_sb[:], in0=dots_sb[:], scalar=1.0, in1=w_sb[:],
                    op0=ALU.bypass, op1=ALU.mult, accum_out=denom_sb[:])

                lnd = wsm((P, 1), "lnd")
                nc.scalar.activation(lnd[:], denom_sb[:], AF.Ln)
                diff = wsm((P, 1), "diff")
                nc.vector.tensor_sub(diff[:], lnd[:], diag_sb[:])
                psf = psB.tile([1, 1], FP, tag="psB", name="ps_loss")
                nc.tensor.matmul(psf[:], diff[:], onesf_col[:])
                res = wsm((1, 1), "res")
                nc.vector.tensor_copy(res[:], psf[:])
                nc.sync.dma_start(out_ext[:], res[:])

    nc.finalize()
    return nc


_NC_CACHE = {}


def _get_nc():
    if "nc" not in _NC_CACHE:
        _NC_CACHE["nc"] = _build()
    return _NC_CACHE["nc"]


def kernel(**inputs):
    inp = {k: np.ascontiguousarray(np.asarray(v, dtype=np.float32))
           for k, v in inputs.items()}
    nc = _get_nc()

    def bf(x):
        return np.ascontiguousarray(x.astype(NPBF))

    rep = {}
    for k in ["fc0_w", "fc1_w", "fc2_w", "agg0_w", "agg1_w", "gcn_w1",
              "gcn_w2", "att_w", "proj_w", "mlp1_w"]:
        rep[k] = bf(inp[k])
    rep["mlp2_w"] = np.ascontiguousarray(inp["mlp2_w"].reshape(16, 1).T)
    rep["fc0_b"] = np.ascontiguousarray(inp["fc0_b"].reshape(4, P).T)
    rep["fc1_b"] = np.ascontiguousarray(inp["fc1_b"].reshape(1, H))
    rep["fc2_b"] = np.ascontiguousarray(inp["fc2_b"].reshape(1, H))
    for k in ["gcn_b1", "gcn_b2", "att_b", "proj_b"]:
        rep[k] = np.ascontiguousarray(inp[k].reshape(E, 1))
    rep["att_vec"] = np.ascontiguousarray(inp["att_vec"].reshape(E, 1))
    rep["mlp1_b"] = np.ascontiguousarray(inp["mlp1_b"].reshape(1, 16))
    rep["mlp2_b"] = np.ascontiguousarray(inp["mlp2_b"].reshape(1, 1))
    rep["sel16"] = bf(
        np.kron(np.eye(16, dtype=np.float32), np.ones((1, P), np.float32)))

    in_maps = []
    for r in range(C):
        rs = slice(r * NL, (r + 1) * NL)
        ms = slice(r * ML, (r + 1) * ML)
        d = dict(rep)
        d["feat0T"] = bf(inp["feat0"][rs].T)
        d["maskT"] = bf(inp["mask_feat"][rs].T)
        d["feat1T"] = bf(inp["feat1"][ms].T)
        d["feat2T"] = bf(inp["feat2"][ms].T)
        d["nei0T"] = bf(inp["nei0"][:, ms].T)
        d["nei1T"] = bf(inp["nei1"][:, ms].T)
        d["adj0T"] = bf(inp["adj0"][rs].T)
        d["adj1T"] = bf(inp["adj1"][rs].T)
        d["madj0T"] = bf(inp["madj0"][rs].T)
        d["madj1T"] = bf(inp["madj1"][rs].T)
        in_maps.append(d)

    trace = bool(int(os.environ.get("KERNEL_TRACE", "0")))
    res = run_bass_kernel_spmd(
        nc, in_maps, core_ids=list(range(C)), trace=trace)
    if trace:
        _NC_CACHE["exec_time_ns"] = res.exec_time_ns
        _NC_CACHE["trace"] = res.instructions_and_trace
    total = sum(float(res.results[r]["out"][0, 0]) for r in range(C))
    return np.float32(total / N)
